# revision 5
# baseline (speedup 1.0000x reference)
"""Causal multi-head attention (CoreAttention) for Trainium2, 8 NeuronCores.

Strategy
--------
The problem is 64 independent (batch, head) attention instances of
[sq=2048, hn=64].  We shard them 8-per-core (tensor-parallel over heads x
data-parallel over batch) -- fully data parallel, no collectives.

Host-side (shard prep): Q and K are pre-transposed to [pair, hn, sq] and V
gets a ones-column appended ([pair, sq, 65]) so that on-chip:

  S^T[sk_blk, q]   = matmul(lhsT=K^T[:, blk], rhs=Q^T[:, q_chunk])    (K=hn=64)
  E = exp(S^T / 8) via ScalarE straight out of PSUM
  causal triangle of diagonal blocks zeroed with one DVE multiply
  ctx^T[65, q]    += matmul(lhsT=[V|1][blk], rhs=E[blk])              (K=sk=128)

ctx^T row 64 is the softmax denominator; the final division and the
transpose back to [sq, b, np*hn] happen on the host.  Skipping the max
subtraction is safe: scores/8 ~ N(0,1), |s|<~7, exp is far from overflow,
and softmax is shift invariant so the result matches the reference.

Causality: sk blocks strictly above the diagonal are never computed;
diagonal-band matmuls restrict their q columns to the valid range, and
their score spans are COMPACTED side by side in the PSUM staging tile so
each group of blocks needs exactly one ScalarE exp instruction (the
per-instruction ACT overhead of ~352 cycles is the #1 cost at this size).
"""

import os
import sys

import numpy as np

if "/opt/trn_rl_repo" not in sys.path:
    sys.path.insert(0, "/opt/trn_rl_repo")

import concourse.bass as bass
import concourse.mybir as mybir
import concourse.tile as tile
from concourse import bacc

SQ, B, NP, HN = 2048, 4, 16, 64
N_CORES = 8
PAIRS_TOTAL = B * NP            # 64 (b, h) instances
PAIRS = PAIRS_TOTAL // N_CORES  # 8 per core
CH = 512                        # q chunk (one PSUM bank of fp32)
NBLK = SQ // 128                # 16 sk blocks
GROUP = 3                       # sk blocks per PSUM score-staging tile
F32 = mybir.dt.float32
MM_DTYPE = mybir.dt.float16     # matmul operand dtype (weights + streams)


def build_attention_module(
    pairs: int = PAIRS,
    nchunks: int = SQ // CH,
    mask: bool = True,
    repeat: int = 1,
    mm_dtype=None,
    loop_n: int | None = None,
) -> bass.Bass:
    MMDT = MM_DTYPE if mm_dtype is None else mm_dtype
    nc = bacc.Bacc(trn_type="TRN2")
    qt = nc.dram_tensor("qt", [pairs, HN, SQ], MMDT, kind="ExternalInput")
    kt = nc.dram_tensor("kt", [pairs, HN, SQ], MMDT, kind="ExternalInput")
    v1 = nc.dram_tensor("v1", [pairs, SQ, HN + 1], MMDT, kind="ExternalInput")
    tri = nc.dram_tensor("tri", [128, 128], MMDT, kind="ExternalInput")
    out = nc.dram_tensor("ctxu", [pairs, HN + 1, SQ], F32, kind="ExternalOutput")

    with tile.TileContext(nc) as tc:
        with (
            tc.tile_pool(name="consts", bufs=1) as consts,
            tc.tile_pool(name="qk", bufs=2) as qkpool,
            tc.tile_pool(name="vp", bufs=2) as vpool,
            tc.tile_pool(name="exps", bufs=3) as epool,
            tc.tile_pool(name="outs", bufs=2) as opool,
            tc.tile_pool(name="spsum", bufs=2, space="PSUM") as spool,
            tc.tile_pool(name="cpsum", bufs=2, space="PSUM") as cpool,
        ):
            tri_t = consts.tile([128, 128], MMDT)
            nc.sync.dma_start(tri_t[:], tri[:])

            import contextlib

            loop_cm = (
                tc.For_i(0, loop_n, 1)
                if loop_n is not None
                else contextlib.nullcontext()
            )
            with loop_cm:
                _pair_body(
                    nc, pairs, repeat, nchunks, mask,
                    qt, kt, v1, out,
                    qkpool, vpool, epool, opool, spool, cpool, tri_t,
                )
    nc.finalize()
    return nc


def _pair_body(
    nc, pairs, repeat, nchunks, mask,
    qt, kt, v1, out,
    qkpool, vpool, epool, opool, spool, cpool, tri_t,
):
    MMDT = tri_t.dtype

    def emit_qk_group(s_ps, grp, j, qt_t, kt_t):
        # Scores for the blocks of one group, COMPACTED side by side:
        # slot for block i starts at the cumulative width so the whole
        # group is one contiguous span (one exp instruction, no garbage).
        placements = []
        c0 = 0
        for i in grp:
            off = max(0, 128 * i - CH * j)
            width = CH - off
            nc.tensor.matmul(
                s_ps[:, c0 : c0 + width],
                lhsT=kt_t[:, 128 * i : 128 * (i + 1)],
                rhs=qt_t[:, CH * j + off : CH * (j + 1)],
                start=True,
                stop=True,
            )
            placements.append((i, c0, off, width))
            c0 += width
        return placements, c0

    def plan_groups(j, nblocks):
        # Pack blocks into staging tiles of GROUP*CH elements.  A matmul
        # output may not cross a 512-elem PSUM bank boundary, so diagonal
        # blocks (widths 512/384/256/128) are ordered 512,384,128,256 --
        # with that order every span lands inside a bank.
        full = [i for i in range(nblocks) if 128 * i < CH * j]
        diag = [i for i in range(nblocks) if 128 * i >= CH * j]
        order = full + [diag[0], diag[1], diag[3], diag[2]]
        cap = GROUP * CH
        groups, cur, c0 = [], [], 0
        for i in order:
            off = max(0, 128 * i - CH * j)
            width = CH - off
            bank_rem = (-c0) % CH or CH
            if c0 + width > cap or (width > bank_rem):
                groups.append(cur)
                cur, c0 = [], 0
            cur.append(i)
            c0 += width
        if cur:
            groups.append(cur)
        return groups

    def load_pair(p, first):
        qt_t = qkpool.tile([HN, SQ], MMDT, tag="qt", name="qt_t")
        kt_t = qkpool.tile([HN, SQ], MMDT, tag="kt", name="kt_t")
        v1_t = vpool.tile([128, NBLK, HN + 1], MMDT, tag="v1", name="v1_t")
        if first:
            # split the very first loads so the first score group's
            # data lands early (cuts the pipeline-fill stall)
            kb = GROUP * 128
            nc.sync.dma_start(qt_t[:, :CH], qt[p][:, :CH])
            nc.sync.dma_start(kt_t[:, :kb], kt[p][:, :kb])
            nc.sync.dma_start(qt_t[:, CH:], qt[p][:, CH:])
            nc.sync.dma_start(kt_t[:, kb:], kt[p][:, kb:])
        else:
            nc.sync.dma_start(qt_t[:], qt[p])
            nc.sync.dma_start(kt_t[:], kt[p])
        nc.sync.dma_start(v1_t[:], v1[p].rearrange("(i s) c -> s i c", s=128))
        return qt_t, kt_t, v1_t

    seq = [p for _ in range(repeat) for p in range(pairs)]
    for pi, p in enumerate(seq):
        qt_t, kt_t, v1_t = load_pair(p, pi == 0)
        out_sb = opool.tile([HN + 1, SQ], F32, tag="osb")

        for j in range(nchunks):  # q chunk
            nblocks = (j + 1) * (CH // 128)  # causal: sk blocks needed
            ctx_ps = cpool.tile([HN + 1, CH], F32, tag="ctx")
            groups = plan_groups(j, nblocks)
            pv_seq = [i for grp in groups for i in grp]
            first_pv, last_pv = pv_seq[0], pv_seq[-1]
            for grp in groups:
                s_ps = spool.tile([128, GROUP * CH], F32, tag="s")
                placements, total_w = emit_qk_group(s_ps, grp, j, qt_t, kt_t)
                exps_t = epool.tile([128, GROUP * CH], MMDT, tag="e")
                nc.scalar.activation(
                    exps_t[:, :total_w],
                    s_ps[:, :total_w],
                    mybir.ActivationFunctionType.Exp,
                    scale=0.125,
                )
                for i, c0, off, width in placements:
                    if mask and 128 * i >= CH * j:
                        # diagonal block: zero the upper triangle
                        nc.vector.tensor_mul(
                            exps_t[:, c0 : c0 + 128],
                            exps_t[:, c0 : c0 + 128],
                            tri_t[:],
                        )
                    nc.tensor.matmul(
                        ctx_ps[:, off:CH],
                        lhsT=v1_t[:, i, :],
                        rhs=exps_t[:, c0 : c0 + width],
                        start=(i == first_pv),
                        stop=(i == last_pv),
                    )
            nc.vector.tensor_copy(out_sb[:, CH * j : CH * (j + 1)], ctx_ps[:])
        nc.sync.dma_start(out[p], out_sb[:])


def prep_inputs(q: np.ndarray, k: np.ndarray, v: np.ndarray, mm_dtype=None):
    """Full [sq, b, np, hn] tensors -> per-pair device layouts."""
    npdt = mybir.dt.np(MM_DTYPE if mm_dtype is None else mm_dtype)
    q = np.asarray(q, dtype=np.float32)
    k = np.asarray(k, dtype=np.float32)
    v = np.asarray(v, dtype=np.float32)
    # [sq, b, np, hn] -> [b*np (pair), hn, sq]
    qt = np.ascontiguousarray(
        q.transpose(1, 2, 3, 0).reshape(PAIRS_TOTAL, HN, SQ).astype(npdt)
    )
    kt = np.ascontiguousarray(
        k.transpose(1, 2, 3, 0).reshape(PAIRS_TOTAL, HN, SQ).astype(npdt)
    )
    # [sq, b, np, hn] -> [pair, sq, hn] with ones column appended
    vr = v.transpose(1, 2, 0, 3).reshape(PAIRS_TOTAL, SQ, HN)
    v1 = np.concatenate(
        [vr, np.ones((PAIRS_TOTAL, SQ, 1), dtype=np.float32)], axis=2
    )
    v1 = np.ascontiguousarray(v1.astype(npdt))
    # exps is [sk (partition), q (free)]; keep iff q >= sk:
    # tri[s, c] = 1 where c >= s, which is exactly np.triu.
    tri = np.ascontiguousarray(np.triu(np.ones((128, 128), dtype=np.float32)).astype(npdt))
    return qt, kt, v1, tri


def postprocess(ctxu: np.ndarray) -> np.ndarray:
    """[pairs_total, 65, sq] unnormalized -> [sq, b, np*hn]."""
    ctx = ctxu[:, :HN, :] / ctxu[:, HN : HN + 1, :]
    # [pair, hn, sq] -> [sq, b, np, hn] -> [sq, b, np*hn]
    ctx = ctx.reshape(B, NP, HN, SQ).transpose(3, 0, 1, 2)
    return np.ascontiguousarray(ctx.reshape(SQ, B, NP * HN)).astype(np.float32)


_NC_CACHE: dict = {}


def kernel(query_layer, key_layer, value_layer, attention_mask=None, **_ignored):
    from concourse.bass_utils import run_bass_kernel_spmd

    qt, kt, v1, tri = prep_inputs(query_layer, key_layer, value_layer)

    if "nc" not in _NC_CACHE:
        _NC_CACHE["nc"] = build_attention_module(PAIRS)
    nc = _NC_CACHE["nc"]

    in_maps = []
    for c in range(N_CORES):
        sl = slice(c * PAIRS, (c + 1) * PAIRS)
        in_maps.append(
            {"qt": qt[sl], "kt": kt[sl], "v1": v1[sl], "tri": tri}
        )
    try:
        res = run_bass_kernel_spmd(nc, in_maps, core_ids=list(range(N_CORES)))
    except Exception:
        # rare transient device error: retry once
        res = run_bass_kernel_spmd(nc, in_maps, core_ids=list(range(N_CORES)))
    ctxu = np.concatenate([r["ctxu"] for r in res.results], axis=0)
    return postprocess(ctxu)


# revision 8
# speedup vs baseline: 1.0473x; 1.0473x over previous
"""Causal multi-head attention (CoreAttention) for Trainium2, 8 NeuronCores.

Strategy
--------
The problem is 64 independent (batch, head) attention instances of
[sq=2048, hn=64].  We shard them 8-per-core (tensor-parallel over heads x
data-parallel over batch) -- fully data parallel, no collectives.

Host-side (shard prep): Q and K are pre-transposed to [pair, hn, sq] and V
gets a ones-column appended ([pair, sq, 65]) so that on-chip:

  S^T[sk_blk, q]   = matmul(lhsT=K^T[:, blk], rhs=Q^T[:, q_chunk])    (K=hn=64)
  E = exp(S^T / 8) via ScalarE straight out of PSUM
  causal triangle of diagonal blocks zeroed with one DVE multiply
  ctx^T[65, q]    += matmul(lhsT=[V|1][blk], rhs=E[blk])              (K=sk=128)

ctx^T row 64 is the softmax denominator; the final division and the
transpose back to [sq, b, np*hn] happen on the host.  Skipping the max
subtraction is safe: scores/8 ~ N(0,1), |s|<~7, exp is far from overflow,
and softmax is shift invariant so the result matches the reference.

Causality: sk blocks strictly above the diagonal are never computed;
diagonal-band matmuls restrict their q columns to the valid range, and
their score spans are COMPACTED side by side in the PSUM staging tile so
each group of blocks needs exactly one ScalarE exp instruction (the
per-instruction ACT overhead of ~352 cycles is the #1 cost at this size).
"""

import os
import sys

import numpy as np

if "/opt/trn_rl_repo" not in sys.path:
    sys.path.insert(0, "/opt/trn_rl_repo")

import concourse.bass as bass
import concourse.mybir as mybir
import concourse.tile as tile
from concourse import bacc

SQ, B, NP, HN = 2048, 4, 16, 64
N_CORES = 8
PAIRS_TOTAL = B * NP            # 64 (b, h) instances
PAIRS = PAIRS_TOTAL // N_CORES  # 8 per core
CH = 512                        # q chunk (one PSUM bank of fp32)
NBLK = SQ // 128                # 16 sk blocks
GROUP = 3                       # sk blocks per PSUM score-staging tile
F32 = mybir.dt.float32
MM_DTYPE = mybir.dt.float16     # matmul operand dtype (weights + streams)


def build_attention_module(
    pairs: int = PAIRS,
    nchunks: int = SQ // CH,
    mask: bool = True,
    repeat: int = 1,
    mm_dtype=None,
    loop_n: int | None = None,
) -> bass.Bass:
    MMDT = MM_DTYPE if mm_dtype is None else mm_dtype
    nc = bacc.Bacc(trn_type="TRN2")
    qt = nc.dram_tensor("qt", [pairs, HN, SQ], MMDT, kind="ExternalInput")
    kt = nc.dram_tensor("kt", [pairs, HN, SQ], MMDT, kind="ExternalInput")
    v1 = nc.dram_tensor("v1", [pairs, SQ, HN + 1], MMDT, kind="ExternalInput")
    tri = nc.dram_tensor("tri", [128, 128], MMDT, kind="ExternalInput")
    out = nc.dram_tensor("ctxu", [pairs, HN + 1, SQ], F32, kind="ExternalOutput")

    with tile.TileContext(nc) as tc:
        with (
            tc.tile_pool(name="consts", bufs=1) as consts,
            tc.tile_pool(name="qk", bufs=2) as qkpool,
            tc.tile_pool(name="vp", bufs=2) as vpool,
            tc.tile_pool(name="exps", bufs=3) as epool,
            tc.tile_pool(name="outs", bufs=2) as opool,
            tc.tile_pool(name="spsum", bufs=2, space="PSUM") as spool,
            tc.tile_pool(name="cpsum", bufs=2, space="PSUM") as cpool,
        ):
            tri_t = consts.tile([128, 128], MMDT)
            nc.sync.dma_start(tri_t[:], tri[:])

            import contextlib

            loop_cm = (
                tc.For_i(0, loop_n, 1)
                if loop_n is not None
                else contextlib.nullcontext()
            )
            with loop_cm:
                _pair_body(
                    nc, pairs, repeat, nchunks, mask,
                    qt, kt, v1, out,
                    qkpool, vpool, epool, opool, spool, cpool, tri_t,
                )
    nc.finalize()
    return nc


def _pair_body(
    nc, pairs, repeat, nchunks, mask,
    qt, kt, v1, out,
    qkpool, vpool, epool, opool, spool, cpool, tri_t,
):
    MMDT = tri_t.dtype

    def emit_qk_group(s_ps, grp, j, qt_t, kt_t):
        # Scores for the blocks of one group, COMPACTED side by side:
        # slot for block i starts at the cumulative width so the whole
        # group is one contiguous span (one exp instruction, no garbage).
        placements = []
        c0 = 0
        for i in grp:
            off = max(0, 128 * i - CH * j)
            width = CH - off
            nc.tensor.matmul(
                s_ps[:, c0 : c0 + width],
                lhsT=kt_t[:, 128 * i : 128 * (i + 1)],
                rhs=qt_t[:, CH * j + off : CH * (j + 1)],
                start=True,
                stop=True,
            )
            placements.append((i, c0, off, width))
            c0 += width
        return placements, c0

    def plan_groups(j, nblocks):
        # Pack blocks into staging tiles of GROUP*CH elements.  A matmul
        # output may not cross a 512-elem PSUM bank boundary, so diagonal
        # blocks (widths 512/384/256/128) are ordered 512,384,128,256 --
        # with that order every span lands inside a bank.
        full = [i for i in range(nblocks) if 128 * i < CH * j]
        diag = [i for i in range(nblocks) if 128 * i >= CH * j]
        order = full + [diag[0], diag[1], diag[3], diag[2]]
        cap = GROUP * CH
        groups, cur, c0 = [], [], 0
        for i in order:
            off = max(0, 128 * i - CH * j)
            width = CH - off
            bank_rem = (-c0) % CH or CH
            if c0 + width > cap or (width > bank_rem):
                groups.append(cur)
                cur, c0 = [], 0
            cur.append(i)
            c0 += width
        if cur:
            groups.append(cur)
        return groups

    def load_pair(p, first):
        qt_t = qkpool.tile([HN, SQ], MMDT, tag="qt", name="qt_t")
        kt_t = qkpool.tile([HN, SQ], MMDT, tag="kt", name="kt_t")
        v1_t = vpool.tile([128, NBLK, HN + 1], MMDT, tag="v1", name="v1_t")
        if first:
            # split the very first loads so the first score group's
            # data lands early (cuts the pipeline-fill stall)
            kb = GROUP * 128
            nc.sync.dma_start(qt_t[:, :CH], qt[p][:, :CH])
            nc.sync.dma_start(kt_t[:, :kb], kt[p][:, :kb])
            nc.sync.dma_start(qt_t[:, CH:], qt[p][:, CH:])
            nc.sync.dma_start(kt_t[:, kb:], kt[p][:, kb:])
        else:
            nc.sync.dma_start(qt_t[:], qt[p])
            nc.sync.dma_start(kt_t[:], kt[p])
        nc.sync.dma_start(v1_t[:], v1[p].rearrange("(i s) c -> s i c", s=128))
        return qt_t, kt_t, v1_t

    seq = [p for _ in range(repeat) for p in range(pairs)]

    # Build the flat list of group tasks.  Per-(pair,chunk) bookkeeping is
    # attached to the FIRST and LAST group of each chunk/pair so tile
    # allocation and copies/stores happen at the right flat positions.
    tasks = []
    for pi, p in enumerate(seq):
        for j in range(nchunks):
            nblocks = (j + 1) * (CH // 128)
            groups = plan_groups(j, nblocks)
            pv_seq = [i for grp in groups for i in grp]
            for gi, grp in enumerate(groups):
                tasks.append(
                    dict(
                        pi=pi, p=p, j=j, grp=grp,
                        first_of_chunk=(gi == 0),
                        last_of_chunk=(gi == len(groups) - 1),
                        first_of_pair=(gi == 0 and j == 0),
                        last_of_pair=(gi == len(groups) - 1 and j == nchunks - 1),
                        first_pv=pv_seq[0],
                        last_pv=pv_seq[-1],
                    )
                )

    # Software pipeline: PV of group g is emitted after QK of group g+1 so
    # the PE streams scores while ACT runs exp(g); the ctx->SBUF copy of a
    # chunk is emitted one group later still so DVE's triangle multiplies
    # never queue behind it.
    state: dict = {}
    pend_pv: list = []
    pend_copy: list = []

    def emit_pv(t):
        for i, c0, off, width in t["placements"]:
            nc.tensor.matmul(
                t["ctx_ps"][:, off:CH],
                lhsT=t["v1_t"][:, i, :],
                rhs=t["exps_t"][:, c0 : c0 + width],
                start=(i == t["first_pv"]),
                stop=(i == t["last_pv"]),
            )
        if t["last_of_chunk"]:
            pend_copy.append(t)

    def emit_copy(t):
        nc.vector.tensor_copy(
            t["out_sb"][:, CH * t["j"] : CH * (t["j"] + 1)], t["ctx_ps"][:]
        )
        if t["last_of_pair"]:
            nc.sync.dma_start(out[t["p"]], t["out_sb"][:])

    for t in tasks:
        if t["first_of_pair"]:
            # tiles for this pair were prefetched one pair ago; issue the
            # NEXT pair's loads now so its QK never waits on DMA
            if t["pi"] == 0:
                state["tiles"] = load_pair(t["p"], True)
            else:
                state["tiles"] = state.pop("tiles_next")
            if t["pi"] + 1 < len(seq):
                state["tiles_next"] = load_pair(seq[t["pi"] + 1], False)
            state["out_sb"] = opool.tile([HN + 1, SQ], F32, tag="osb", name="out_sb")
        qt_t, kt_t, v1_t = state["tiles"]
        if t["first_of_chunk"]:
            state["ctx_ps"] = cpool.tile([HN + 1, CH], F32, tag="ctx", name="ctx_ps")
        t["v1_t"], t["ctx_ps"], t["out_sb"] = v1_t, state["ctx_ps"], state["out_sb"]

        s_ps = spool.tile([128, GROUP * CH], F32, tag="s")
        t["placements"], total_w = emit_qk_group(s_ps, t["grp"], t["j"], qt_t, kt_t)
        t["exps_t"] = epool.tile([128, GROUP * CH], MMDT, tag="e", name="exps_t")
        nc.scalar.activation(
            t["exps_t"][:, :total_w],
            s_ps[:, :total_w],
            mybir.ActivationFunctionType.Exp,
            scale=0.125,
        )
        for i, c0, off, width in t["placements"]:
            if mask and 128 * i >= CH * t["j"]:
                # diagonal block: zero the upper triangle
                nc.vector.tensor_mul(
                    t["exps_t"][:, c0 : c0 + 128],
                    t["exps_t"][:, c0 : c0 + 128],
                    tri_t[:],
                )
        if pend_pv:
            emit_pv(pend_pv.pop())
        while pend_copy:
            emit_copy(pend_copy.pop(0))
        pend_pv.append(t)

    while pend_pv:
        emit_pv(pend_pv.pop())
    while pend_copy:
        emit_copy(pend_copy.pop(0))


def prep_inputs(q: np.ndarray, k: np.ndarray, v: np.ndarray, mm_dtype=None):
    """Full [sq, b, np, hn] tensors -> per-pair device layouts."""
    npdt = mybir.dt.np(MM_DTYPE if mm_dtype is None else mm_dtype)
    q = np.asarray(q, dtype=np.float32)
    k = np.asarray(k, dtype=np.float32)
    v = np.asarray(v, dtype=np.float32)
    # [sq, b, np, hn] -> [b*np (pair), hn, sq]
    qt = np.ascontiguousarray(
        q.transpose(1, 2, 3, 0).reshape(PAIRS_TOTAL, HN, SQ).astype(npdt)
    )
    kt = np.ascontiguousarray(
        k.transpose(1, 2, 3, 0).reshape(PAIRS_TOTAL, HN, SQ).astype(npdt)
    )
    # [sq, b, np, hn] -> [pair, sq, hn] with ones column appended
    vr = v.transpose(1, 2, 0, 3).reshape(PAIRS_TOTAL, SQ, HN)
    v1 = np.concatenate(
        [vr, np.ones((PAIRS_TOTAL, SQ, 1), dtype=np.float32)], axis=2
    )
    v1 = np.ascontiguousarray(v1.astype(npdt))
    # exps is [sk (partition), q (free)]; keep iff q >= sk:
    # tri[s, c] = 1 where c >= s, which is exactly np.triu.
    tri = np.ascontiguousarray(np.triu(np.ones((128, 128), dtype=np.float32)).astype(npdt))
    return qt, kt, v1, tri


def postprocess(ctxu: np.ndarray) -> np.ndarray:
    """[pairs_total, 65, sq] unnormalized -> [sq, b, np*hn]."""
    ctx = ctxu[:, :HN, :] / ctxu[:, HN : HN + 1, :]
    # [pair, hn, sq] -> [sq, b, np, hn] -> [sq, b, np*hn]
    ctx = ctx.reshape(B, NP, HN, SQ).transpose(3, 0, 1, 2)
    return np.ascontiguousarray(ctx.reshape(SQ, B, NP * HN)).astype(np.float32)


_NC_CACHE: dict = {}


def kernel(query_layer, key_layer, value_layer, attention_mask=None, **_ignored):
    from concourse.bass_utils import run_bass_kernel_spmd

    qt, kt, v1, tri = prep_inputs(query_layer, key_layer, value_layer)

    if "nc" not in _NC_CACHE:
        _NC_CACHE["nc"] = build_attention_module(PAIRS)
    nc = _NC_CACHE["nc"]

    in_maps = []
    for c in range(N_CORES):
        sl = slice(c * PAIRS, (c + 1) * PAIRS)
        in_maps.append(
            {"qt": qt[sl], "kt": kt[sl], "v1": v1[sl], "tri": tri}
        )
    try:
        res = run_bass_kernel_spmd(nc, in_maps, core_ids=list(range(N_CORES)))
    except Exception:
        # rare transient device error: retry once
        res = run_bass_kernel_spmd(nc, in_maps, core_ids=list(range(N_CORES)))
    ctxu = np.concatenate([r["ctxu"] for r in res.results], axis=0)
    return postprocess(ctxu)


# revision 11
# speedup vs baseline: 1.6734x; 1.5979x over previous
"""Causal multi-head attention (CoreAttention) for Trainium2, 8 NeuronCores.

Strategy
--------
The problem is 64 independent (batch, head) attention instances of
[sq=2048, hn=64].  We shard them 8-per-core (tensor-parallel over heads x
data-parallel over batch) -- fully data parallel, no collectives.

Host-side (shard prep): Q and K are pre-transposed to [pair, hn, sq] and V
gets a ones-column appended ([pair, sq, 65]) so that on-chip:

  S^T[sk_blk, q]   = matmul(lhsT=K^T[:, blk], rhs=Q^T[:, q_chunk])    (K=hn=64)
  E = exp(S^T / 8) via ScalarE straight out of PSUM
  causal triangle of diagonal blocks zeroed with one DVE multiply
  ctx^T[65, q]    += matmul(lhsT=[V|1][blk], rhs=E[blk])              (K=sk=128)

ctx^T row 64 is the softmax denominator; the final division and the
transpose back to [sq, b, np*hn] happen on the host.  Skipping the max
subtraction is safe: scores/8 ~ N(0,1), |s|<~7, exp is far from overflow,
and softmax is shift invariant so the result matches the reference.

Causality: sk blocks strictly above the diagonal are never computed;
diagonal-band matmuls restrict their q columns to the valid range, and
their score spans are COMPACTED side by side in the PSUM staging tile so
each group of blocks needs exactly one ScalarE exp instruction (the
per-instruction ACT overhead of ~352 cycles is the #1 cost at this size).
"""

import os
import sys

import numpy as np

if "/opt/trn_rl_repo" not in sys.path:
    sys.path.insert(0, "/opt/trn_rl_repo")

import concourse.bass as bass
import concourse.mybir as mybir
import concourse.tile as tile
from concourse import bacc

SQ, B, NP, HN = 2048, 4, 16, 64
N_CORES = 8
PAIRS_TOTAL = B * NP            # 64 (b, h) instances
PAIRS = PAIRS_TOTAL // N_CORES  # 8 per core
CH = 512                        # q chunk (one PSUM bank of fp32)
NBLK = SQ // 128                # 16 sk blocks
GROUP = 3                       # sk blocks per PSUM score-staging tile
F32 = mybir.dt.float32
MM_DTYPE = mybir.dt.float16     # matmul operand dtype (weights + streams)


def build_attention_module(
    pairs: int = PAIRS,
    nchunks: int = SQ // CH,
    mask: bool = True,
    repeat: int = 1,
    mm_dtype=None,
    loop_n: int | None = None,
) -> bass.Bass:
    MMDT = MM_DTYPE if mm_dtype is None else mm_dtype
    nc = bacc.Bacc(trn_type="TRN2")
    qt = nc.dram_tensor("qt", [pairs, HN, SQ], MMDT, kind="ExternalInput")
    kt = nc.dram_tensor("kt", [pairs, HN, SQ], MMDT, kind="ExternalInput")
    v1 = nc.dram_tensor("v1", [pairs, SQ, HN + 1], MMDT, kind="ExternalInput")
    tri = nc.dram_tensor("tri", [128, 128], MMDT, kind="ExternalInput")
    out = nc.dram_tensor("ctxu", [pairs, HN + 1, SQ], F32, kind="ExternalOutput")

    with tile.TileContext(nc) as tc:
        with (
            tc.tile_pool(name="consts", bufs=1) as consts,
            tc.tile_pool(name="qk", bufs=2) as qkpool,
            tc.tile_pool(name="vp", bufs=2) as vpool,
            tc.tile_pool(name="exps", bufs=4) as epool,
            tc.tile_pool(name="outs", bufs=3) as opool,
            tc.tile_pool(name="spsum", bufs=2, space="PSUM") as spool,
            tc.tile_pool(name="cpsum", bufs=2, space="PSUM") as cpool,
        ):
            tri_t = consts.tile([128, 128], MMDT)
            nc.sync.dma_start(tri_t[:], tri[:])

            import contextlib

            loop_cm = (
                tc.For_i(0, loop_n, 1)
                if loop_n is not None
                else contextlib.nullcontext()
            )
            with loop_cm:
                _pair_body(
                    nc, pairs, repeat, nchunks, mask,
                    qt, kt, v1, out,
                    qkpool, vpool, epool, opool, spool, cpool, tri_t,
                )
    nc.finalize()
    return nc


def _pair_body(
    nc, pairs, repeat, nchunks, mask,
    qt, kt, v1, out,
    qkpool, vpool, epool, opool, spool, cpool, tri_t,
):
    MMDT = tri_t.dtype

    def emit_qk_group(s_ps, grp, j, qt_t, kt_t):
        # Scores for the blocks of one group, COMPACTED side by side:
        # slot for block i starts at the cumulative width so the whole
        # group is one contiguous span (one exp instruction, no garbage).
        placements = []
        c0 = 0
        for i in grp:
            off = max(0, 128 * i - CH * j)
            width = CH - off
            nc.tensor.matmul(
                s_ps[:, c0 : c0 + width],
                lhsT=kt_t[:, 128 * i : 128 * (i + 1)],
                rhs=qt_t[:, CH * j + off : CH * (j + 1)],
                start=True,
                stop=True,
            )
            placements.append((i, c0, off, width))
            c0 += width
        return placements, c0

    def plan_groups(j, nblocks):
        # Pack blocks into staging tiles of GROUP*CH elements.  A matmul
        # output may not cross a 512-elem PSUM bank boundary, so diagonal
        # blocks (widths 512/384/256/128) are ordered 512,384,128,256 --
        # with that order every span lands inside a bank.
        full = [i for i in range(nblocks) if 128 * i < CH * j]
        diag = [i for i in range(nblocks) if 128 * i >= CH * j]
        order = full + [diag[0], diag[1], diag[3], diag[2]]
        cap = GROUP * CH
        groups, cur, c0 = [], [], 0
        for i in order:
            off = max(0, 128 * i - CH * j)
            width = CH - off
            bank_rem = (-c0) % CH or CH
            if c0 + width > cap or (width > bank_rem):
                groups.append(cur)
                cur, c0 = [], 0
            cur.append(i)
            c0 += width
        if cur:
            groups.append(cur)
        return groups

    def load_pair(p, first):
        qt_t = qkpool.tile([HN, SQ], MMDT, tag="qt", name="qt_t")
        kt_t = qkpool.tile([HN, SQ], MMDT, tag="kt", name="kt_t")
        v1_t = vpool.tile([128, NBLK, HN + 1], MMDT, tag="v1", name="v1_t")
        if first:
            # split the very first loads so the first score group's
            # data lands early (cuts the pipeline-fill stall)
            kb = 512
            nc.sync.dma_start(qt_t[:, :CH], qt[p][:, :CH])
            nc.sync.dma_start(kt_t[:, :kb], kt[p][:, :kb])
            nc.sync.dma_start(qt_t[:, CH:], qt[p][:, CH:])
            nc.sync.dma_start(kt_t[:, kb:], kt[p][:, kb:])
        else:
            nc.sync.dma_start(qt_t[:], qt[p])
            nc.sync.dma_start(kt_t[:], kt[p])
        nc.sync.dma_start(v1_t[:], v1[p].rearrange("(i s) c -> s i c", s=128))
        return qt_t, kt_t, v1_t

    seq = [p for _ in range(repeat) for p in range(pairs)]

    # Build the flat list of group tasks.  Per-(pair,chunk) bookkeeping is
    # attached to the FIRST and LAST group of each chunk/pair so tile
    # allocation and copies/stores happen at the right flat positions.
    tasks = []
    for pi, p in enumerate(seq):
        for j in range(nchunks):
            nblocks = (j + 1) * (CH // 128)
            groups = plan_groups(j, nblocks)
            pv_seq = [i for grp in groups for i in grp]
            for gi, grp in enumerate(groups):
                tasks.append(
                    dict(
                        pi=pi, p=p, j=j, grp=grp,
                        first_of_chunk=(gi == 0),
                        last_of_chunk=(gi == len(groups) - 1),
                        first_of_pair=(gi == 0 and j == 0),
                        last_of_pair=(gi == len(groups) - 1 and j == nchunks - 1),
                        first_pv=pv_seq[0],
                        last_pv=pv_seq[-1],
                    )
                )

    # Software pipeline: PV of group g is emitted after QK of group g+2 so
    # the in-order PE never reaches a PV whose exp hasn't long finished;
    # each chunk's ctx is DMA'd straight from PSUM (no SBUF copy).
    PV_DEPTH = 2
    state: dict = {}
    pend_pv: list = []

    def emit_pv(t):
        for i, c0, off, width in t["placements"]:
            nc.tensor.matmul(
                t["ctx_ps"][:, off:CH],
                lhsT=t["v1_t"][:, i, :],
                rhs=t["exps_t"][:, c0 : c0 + width],
                start=(i == t["first_pv"]),
                stop=(i == t["last_pv"]),
            )
        if t["last_of_chunk"]:
            j = t["j"]
            osb = opool.tile([HN + 1, CH], F32, tag="osb", name="osb")
            nc.vector.tensor_copy(osb[:], t["ctx_ps"][:])
            nc.sync.dma_start(out[t["p"]][:, CH * j : CH * (j + 1)], osb[:])

    for t in tasks:
        if t["first_of_pair"]:
            # tiles for this pair were prefetched one pair ago; issue the
            # NEXT pair's loads now so its QK never waits on DMA
            if t["pi"] == 0:
                state["tiles"] = load_pair(t["p"], True)
            else:
                state["tiles"] = state.pop("tiles_next")
            if t["pi"] + 1 < len(seq):
                state["tiles_next"] = load_pair(seq[t["pi"] + 1], False)
        qt_t, kt_t, v1_t = state["tiles"]
        if t["first_of_chunk"]:
            state["ctx_ps"] = cpool.tile([HN + 1, CH], F32, tag="ctx", name="ctx_ps")
        t["v1_t"], t["ctx_ps"] = v1_t, state["ctx_ps"]

        s_ps = spool.tile([128, GROUP * CH], F32, tag="s")
        t["placements"], total_w = emit_qk_group(s_ps, t["grp"], t["j"], qt_t, kt_t)
        t["exps_t"] = epool.tile([128, GROUP * CH], MMDT, tag="e", name="exps_t")
        nc.scalar.activation(
            t["exps_t"][:, :total_w],
            s_ps[:, :total_w],
            mybir.ActivationFunctionType.Exp,
            scale=0.125,
        )
        for i, c0, off, width in t["placements"]:
            if mask and 128 * i >= CH * t["j"]:
                # diagonal block: zero the upper triangle
                nc.vector.tensor_mul(
                    t["exps_t"][:, c0 : c0 + 128],
                    t["exps_t"][:, c0 : c0 + 128],
                    tri_t[:],
                )
        if len(pend_pv) >= PV_DEPTH:
            emit_pv(pend_pv.pop(0))
        pend_pv.append(t)

    while pend_pv:
        emit_pv(pend_pv.pop(0))


def prep_inputs(q: np.ndarray, k: np.ndarray, v: np.ndarray, mm_dtype=None):
    """Full [sq, b, np, hn] tensors -> per-pair device layouts."""
    npdt = mybir.dt.np(MM_DTYPE if mm_dtype is None else mm_dtype)
    q = np.asarray(q, dtype=np.float32)
    k = np.asarray(k, dtype=np.float32)
    v = np.asarray(v, dtype=np.float32)
    # [sq, b, np, hn] -> [b*np (pair), hn, sq]
    qt = np.ascontiguousarray(
        q.transpose(1, 2, 3, 0).reshape(PAIRS_TOTAL, HN, SQ).astype(npdt)
    )
    kt = np.ascontiguousarray(
        k.transpose(1, 2, 3, 0).reshape(PAIRS_TOTAL, HN, SQ).astype(npdt)
    )
    # [sq, b, np, hn] -> [pair, sq, hn] with ones column appended
    vr = v.transpose(1, 2, 0, 3).reshape(PAIRS_TOTAL, SQ, HN)
    v1 = np.concatenate(
        [vr, np.ones((PAIRS_TOTAL, SQ, 1), dtype=np.float32)], axis=2
    )
    v1 = np.ascontiguousarray(v1.astype(npdt))
    # exps is [sk (partition), q (free)]; keep iff q >= sk:
    # tri[s, c] = 1 where c >= s, which is exactly np.triu.
    tri = np.ascontiguousarray(np.triu(np.ones((128, 128), dtype=np.float32)).astype(npdt))
    return qt, kt, v1, tri


def postprocess(ctxu: np.ndarray) -> np.ndarray:
    """[pairs_total, 65, sq] unnormalized -> [sq, b, np*hn]."""
    ctx = ctxu[:, :HN, :] / ctxu[:, HN : HN + 1, :]
    # [pair, hn, sq] -> [sq, b, np, hn] -> [sq, b, np*hn]
    ctx = ctx.reshape(B, NP, HN, SQ).transpose(3, 0, 1, 2)
    return np.ascontiguousarray(ctx.reshape(SQ, B, NP * HN)).astype(np.float32)


_NC_CACHE: dict = {}


def kernel(query_layer, key_layer, value_layer, attention_mask=None, **_ignored):
    from concourse.bass_utils import run_bass_kernel_spmd

    qt, kt, v1, tri = prep_inputs(query_layer, key_layer, value_layer)

    if "nc" not in _NC_CACHE:
        _NC_CACHE["nc"] = build_attention_module(PAIRS)
    nc = _NC_CACHE["nc"]

    in_maps = []
    for c in range(N_CORES):
        sl = slice(c * PAIRS, (c + 1) * PAIRS)
        in_maps.append(
            {"qt": qt[sl], "kt": kt[sl], "v1": v1[sl], "tri": tri}
        )
    try:
        res = run_bass_kernel_spmd(nc, in_maps, core_ids=list(range(N_CORES)))
    except Exception:
        # rare transient device error: retry once
        res = run_bass_kernel_spmd(nc, in_maps, core_ids=list(range(N_CORES)))
    ctxu = np.concatenate([r["ctxu"] for r in res.results], axis=0)
    return postprocess(ctxu)
